# revision 40
# baseline (speedup 1.0000x reference)
"""Trainium2 Bass kernel for NeuralGraphOutput (gnn_message_passing).

Math (per sample b):
    out[b, :] = sum_a mask[b,a] * relu(cat(atoms[b,a,:], sum_d bonds[b,a,d,:]) @ W + bias)
    mask[b,a] = any(edges[b,a,:] != -1)

For the fixed seeded inputs, mask is identically 1 (P(all 8 edges == -1) ~
(1/257)^8 per row); the device kernel computes the unmasked sum and the host
subtracts the (normally empty) set of masked-row contributions exactly.

Strategy: pure data parallel over 8 NeuronCores (256 samples each).
Rows (sample, atom pairs) are processed in blocks of 1024, mapped p-major:
row = blk*1024 + p*8 + g  (p = partition, g in [0,8)). This makes the atoms
DMA 2KB-contiguous and the bonds DMA 4KB-contiguous per partition (full DMA
line rate), with only 2 HWDGE dispatches per block.

Per block:
  - DMA atoms -> atile [128, 8*64] f32, bonds -> btile [128, 8*128] f32
  - GpSimd fold1: d-pairs (d, d+4) -> bt2 [128, 8, 4, 16] f32
  - GpSimd fold2: pairs -> cat[:, :, 64:96] f16  (2 groups of 4 d's each;
    W rows 64:80 / 80:96 are both W_bond so the group split is exact)
  - DVE casts atoms -> cat[:, :, 0:64] f16; cat cols 96 (=1.0 bias) and 97
    (=0 pad) are preset once per buffer
  - PE transposes cat[:, g, :] (f16, 1 cyc/row) -> psum_ct [98, 512]
  - DVE drains psum_ct -> catT [98, 512] f16
  - PE main matmuls: lhsT = catT[:, j*128:(j+1)*128] (stationary),
    rhs = W_aug [98, 256] f16 -> psum_fp [128, 4*256] per half-block
  - ACT relu psum_fp -> relu_sb [128, 4, 256] f16
  - PE reduce matmuls: lhsT = sampsel[:, r*16:(r+1)*16] (one-hot sample
    selector, r = blk%4), rhs = relu_sb[:, j, :] -> psum_out [16, 256]
    accumulated over 4 blocks (16 samples)
  - every 4 blocks: DVE drains psum_out -> stage, out DMA via the ACT
    HWDGE queue (keeps the SP queue clear for input loads); both are
    emitted 3 half-blocks late so their waits are already satisfied and
    they never head-of-line-block the DVE/ACT streams

Pipeline skew: transposes+drains run one block ahead of mains/relu, and
reduce matmuls trail by two half-blocks, so the in-order PE stream never
stalls on the ACT relu or the DVE drain.

Output is sample-major [256, 256] f32 per core; host concatenates.
"""

import os
from contextlib import ExitStack

import numpy as np

import concourse.bass as bass
import concourse.mybir as mybir
import concourse.tile as tile
from concourse import masks
from concourse.bass_utils import run_bass_kernel_spmd

# Problem shapes (hardcoded per contract)
B, A, D, FA, FB, FP = 2048, 256, 8, 64, 16, 256
NCORES = 8
P = 128
G = 8                      # rows per partition per block
ROWS = P * G               # rows per block (1024)
KC = 98                    # 64 atoms + 32 bond-halves + 1 bias + 1 pad
BPG = 4                    # blocks accumulated per psum_out group
SPG = 16                   # samples per group (BPG * ROWS // A)

f32 = mybir.dt.float32
f16 = mybir.dt.float16
f8 = mybir.dt.float8e4
i32 = mybir.dt.int32

# fp8e4m3 DoubleRow reduce matmuls: 2 sub-tiles per matmul at 0.5 cyc/row
REDUCE_FP8 = False


def _env(name, default):
    return int(os.environ.get(name, default))

# Set by kernel() after a run; test.py reads exec_time_ns / trace info.
LAST_RESULTS = None


def legalize_waits(nc, max_inline=1):
    """This toolchain's walrus accepts at most one semaphore wait inline per
    instruction (64B Events struct). Tile emits multi-wait sync_info; split
    the surplus into standalone EventSemaphore instructions just before the
    instruction on the same engine queue — identical semantics."""
    f = nc.m.functions[0]
    for bb in f.blocks:
        new = []
        for inst in bb.instructions:
            si = inst.sync_info
            waits = list(si.on_wait) if (si and si.on_wait) else []
            if len(waits) > max_inline:
                keep = waits[-max_inline:]
                moved = waits[:-max_inline]
                for k, w in enumerate(moved):
                    new.append(
                        mybir.InstEventSemaphore(
                            name=f"{inst.name}-prewait{k}",
                            ins=[],
                            outs=[],
                            sync_info=mybir.SyncInfo(on_wait=[w], on_update=[]),
                            engine=inst.engine,
                        )
                    )
                si.on_wait = keep
            new.append(inst)
        bb.instructions[:] = new
    return nc


def build_nc(n_samples_per_core: int, legalize: bool = True, repeat: int = 1) -> bass.Bass:
    """Build the single-core Bass program (same program runs SPMD on all cores).

    repeat > 1 duplicates the whole computation device-side (identical I/O) —
    used only for benchmarking, to amortize host/RPC dispatch overhead."""
    BC = n_samples_per_core
    N = BC * A                      # flat rows per core
    NBLK = N // ROWS                # 1024-row blocks
    NGRP = NBLK // BPG              # psum_out groups (16 samples each)
    assert NBLK % BPG == 0

    nc = bass.Bass()
    atoms_d = nc.dram_tensor("atoms", [N, FA], f32, kind="ExternalInput")
    bonds_d = nc.dram_tensor("bonds", [N, D * FB], f32, kind="ExternalInput")
    # host passes W_aug f16: rows 0:64 = W_atoms, 64:80 = W_bond, 80:96 =
    # W_bond (replicated for the 2 folded bond groups), 96 = bias, 97 = 0
    w_d = nc.dram_tensor("w", [KC, FP], f16, kind="ExternalInput")
    # sampsel[p, r*16+s] = 1 iff s == 4*r + p//32 (one-hot selector; for the
    # fp8 DoubleRow path it is duplicated across the 2 k-planes)
    sel_dt = f8 if REDUCE_FP8 else f16
    sel_cols = BPG * (2 * SPG if REDUCE_FP8 else SPG)
    sel_d = nc.dram_tensor("sampsel", [P, sel_cols], sel_dt, kind="ExternalInput")
    out_d = nc.dram_tensor("out", [BC, FP], f32, kind="ExternalOutput")

    with ExitStack() as ctx:
        tc = ctx.enter_context(tc_ := tile.TileContext(nc))
        singles = ctx.enter_context(tc.tile_pool(name="singles", bufs=1))

        # ---- pools ----
        atp = ctx.enter_context(tc.tile_pool(name="atp", bufs=_env("KATP", 3)))
        btp = ctx.enter_context(tc.tile_pool(name="btp", bufs=_env("KBTP", 3)))
        bt2p = ctx.enter_context(tc.tile_pool(name="bt2p", bufs=_env("KBT2", 3)))
        catTp = ctx.enter_context(tc.tile_pool(name="catTp", bufs=_env("KCTT", 4)))
        relup = ctx.enter_context(tc.tile_pool(name="relup", bufs=_env("KRELU", 4)))
        stagep = ctx.enter_context(tc.tile_pool(name="stagep", bufs=2))
        psct = ctx.enter_context(tc.tile_pool(name="psct", bufs=2, space="PSUM"))
        psfp = ctx.enter_context(tc.tile_pool(name="psfp", bufs=2, space="PSUM"))
        psout = ctx.enter_context(tc.tile_pool(name="psout", bufs=2, space="PSUM"))

        atoms_r = atoms_d[:, :].rearrange("(T p g) f -> T p (g f)", p=P, g=G)
        bonds_r = bonds_d[:, :].rearrange("(T p g) f -> T p (g f)", p=P, g=G)

        # block-0 loads issued before everything else: the first transposes
        # gate the whole pipeline on DMA + both gpsimd folds
        bt0 = btp.tile([P, G, D * FB], f32)
        nc.sync.dma_start(out=bt0[:], in_=bonds_r[0])
        at0 = atp.tile([P, G, FA], f32)
        nc.sync.dma_start(out=at0[:], in_=atoms_r[0])
        prefetched = {0: (at0, bt0)}

        # ---- constants ----
        w_sb = singles.tile([KC, FP], f16)
        nc.sync.dma_start(out=w_sb[:], in_=w_d[:, :])
        sel_sb = singles.tile([P, sel_cols], sel_dt)
        nc.sync.dma_start(out=sel_sb[:], in_=sel_d[:, :])
        # identity built on gpsimd (f32), laundered to f16 via DVE
        identity_src = singles.tile([P, P], f32)
        masks.make_identity(nc, identity_src[:])
        identity = singles.tile([P, P], f16)
        nc.vector.tensor_copy(identity[:], identity_src[:])

        # cat buffers: manual rotation; bias col 96 = 1.0 and pad col 97 = 0.0
        # are preset once per buffer and never rewritten
        NB = _env("KNB", 4)
        cat_bufs = []
        for i in range(NB):
            cb = singles.tile([P, G, KC], f16, name=f"cat{i}")
            nc.vector.memset(cb[:, :, FA + 2 * FB : FA + 2 * FB + 1], 1.0)
            nc.vector.memset(cb[:, :, FA + 2 * FB + 1 : KC], 0.0)
            cat_bufs.append(cb)

        # The PE sequencer is in-order: reduce matmuls for half-block h are
        # emitted two half-blocks late so they never head-of-line-block the
        # next halves' transposes/mains while ACT's relu is still running.
        pending = []        # [(relu, r, h, psum_out, grp_done: grp or None)]
        stage_q = []        # [(countdown_halves, grp, psum_out)]

        def emit_one():
            relu, r, h, pout, grp_done = pending.pop(0)
            if REDUCE_FP8:
                selv = sel_sb.rearrange("p (r e s) -> p r e s", r=BPG, e=2)
                reluv = relu.rearrange("p (u e) o -> p u e o", e=2)
                for j in range(2):
                    t = 4 * r + 2 * h + j
                    nc.tensor.matmul(
                        pout[:, :],
                        lhsT=selv[:, r],
                        rhs=reluv[:, j],
                        start=(t == 0),
                        stop=(t == 4 * BPG - 1),
                        perf_mode=mybir.MatmulPerfMode.DoubleRow,
                    )
            else:
                for j in range(4):
                    t = 8 * r + 4 * h + j
                    nc.tensor.matmul(
                        pout[:, :],
                        lhsT=sel_sb[:, r * SPG : (r + 1) * SPG],
                        rhs=relu[:, j, :],
                        start=(t == 0),
                        stop=(t == 8 * BPG - 1),
                    )
            if grp_done is not None:
                stage_q.append([_env("KSTG", 3), grp_done, pout])

        def tick_stage(force=False):
            # Emit the psum_out drain only once its producer reduces are
            # long finished, so it never head-of-line-blocks the DVE stream;
            # the out DMA goes through SWDGE to keep the SP queue clear.
            for ent in stage_q:
                ent[0] -= 1
            while stage_q and (force or stage_q[0][0] <= 0):
                _, grp, pout = stage_q.pop(0)
                stage = stagep.tile([SPG, FP], f32)
                nc.vector.tensor_copy(stage[:], pout[:])
                nc.scalar.dma_start(
                    out=out_d[grp * SPG : (grp + 1) * SPG, :], in_=stage[:]
                )

        psum_out = None
        catT_q = {}          # blk -> [catT_h0, catT_h1], transposed 1 blk early
        for rep_blk in range(repeat * (NBLK + 1)):
            blk = rep_blk % (NBLK + 1)
            if blk < NBLK:
                cat = cat_bufs[blk % NB]
                if blk in prefetched:
                    atile, btile = prefetched.pop(blk)
                else:
                    btile = btp.tile([P, G, D * FB], f32)
                    nc.sync.dma_start(out=btile[:], in_=bonds_r[blk])
                    atile = atp.tile([P, G, FA], f32)
                    nc.sync.dma_start(out=atile[:], in_=atoms_r[blk])

                # bond fold on gpsimd: (d, d+4) pairs, then pairs again
                bv = btile.rearrange("p g (e x) -> p g e x", e=2)
                bt2 = bt2p.tile([P, G, (D // 2) * FB], f32)
                nc.gpsimd.tensor_tensor(
                    out=bt2[:], in0=bv[:, :, 0], in1=bv[:, :, 1],
                    op=mybir.AluOpType.add,
                )
                b2v = bt2.rearrange("p g (e x) -> p g e x", e=2)
                nc.gpsimd.tensor_tensor(
                    out=cat[:, :, FA : FA + 2 * FB],
                    in0=b2v[:, :, 0], in1=b2v[:, :, 1],
                    op=mybir.AluOpType.add,
                )
                # atoms cast f32 -> f16 into cat
                nc.vector.tensor_scalar(
                    out=cat[:, :, 0:FA], in0=atile[:], scalar1=0.0,
                    scalar2=None, op0=mybir.AluOpType.add,
                )

                # stage A: transpose + drain (one block ahead of stage B, so
                # the DVE drains overlap the previous block's mains/reduces)
                cts = []
                for h in range(2):
                    psum_ct = psct.tile([KC, 4 * P], f16)
                    for j in range(4):
                        nc.tensor.transpose(
                            psum_ct[:, j * P : (j + 1) * P],
                            cat[:, 4 * h + j, :],
                            identity[:],
                        )
                    catT = catTp.tile([KC, 4 * P], f16)
                    nc.vector.tensor_copy(catT[:], psum_ct[:, :])
                    cts.append(catT)
                catT_q[blk] = cts

            # stage B/C: mains + relu for block blk-1, reduces one half late
            mblk = blk - 1
            if mblk < 0:
                continue
            r = mblk % BPG
            if r == 0:
                psum_out = psout.tile([SPG, FP], f32, name="psum_out")
            for h in range(2):
                catT = catT_q[mblk][h]
                psum_fp = psfp.tile([P, 4 * FP], f32)
                for j in range(4):
                    nc.tensor.matmul(
                        psum_fp[:, j * FP : (j + 1) * FP],
                        lhsT=catT[:, j * P : (j + 1) * P],
                        rhs=w_sb[:, :],
                        start=True,
                        stop=True,
                    )
                relu = relup.tile([P, 4, FP], f8 if REDUCE_FP8 else f16)
                pfv = psum_fp[:].rearrange("p (j o) -> p j o", j=4)
                NGA = _env("KNGA", 4)   # relu g-slices on ACT; rest on DVE
                nc.scalar.activation(
                    relu[:, 0:NGA], pfv[:, 0:NGA],
                    mybir.ActivationFunctionType.Relu,
                )
                if NGA < 4:
                    nc.vector.tensor_scalar(
                        out=relu[:, NGA:4], in0=pfv[:, NGA:4], scalar1=0.0,
                        scalar2=None, op0=mybir.AluOpType.max,
                    )
                grp_done = mblk // BPG if (r == BPG - 1 and h == 1) else None
                pending.append((relu, r, h, psum_out, grp_done))
                if len(pending) > _env("KPEND", 2):
                    emit_one()
                tick_stage()
            del catT_q[mblk]
            if blk == NBLK:
                while pending:
                    emit_one()
                tick_stage(force=True)

        while pending:
            emit_one()
        tick_stage(force=True)
    if legalize:
        legalize_waits(nc)
    return nc


def make_w_aug(W, b):
    """Host-side W layout matching cat columns: atoms | bond x2 | bias | 0."""
    W = np.asarray(W, dtype=np.float32)
    b = np.asarray(b, dtype=np.float32).reshape(1, FP)
    rows = [W[0:FA], W[FA : FA + FB], W[FA : FA + FB], b,
            np.zeros((1, FP), np.float32)]
    return np.ascontiguousarray(np.vstack(rows)).astype(np.float16)


def make_sampsel():
    """sampsel[p, r*16+s] = 1 iff s == 4*r + p//32 (fp8 path: duplicated
    across the 2 DoubleRow k-planes)."""
    if REDUCE_FP8:
        sel = np.zeros((P, BPG, 2, SPG), np.float32)
        for p in range(P):
            for r in range(BPG):
                sel[p, r, :, 4 * r + p // 32] = 1.0
        return sel.reshape(P, -1).astype(mybir.dt.np(f8))
    sel = np.zeros((P, BPG * SPG), np.float16)
    for p in range(P):
        for r in range(BPG):
            sel[p, r * SPG + 4 * r + p // 32] = 1.0
    return sel


def _shard_inputs(atoms, bonds, W, b, n_samples_per_core):
    BC = n_samples_per_core
    N = BC * A
    w_aug = make_w_aug(W, b)
    sel = make_sampsel()
    in_maps = []
    for c in range(NCORES):
        sl = slice(c * BC, (c + 1) * BC)
        in_maps.append(
            {
                "atoms": np.ascontiguousarray(
                    np.asarray(atoms[sl], dtype=np.float32).reshape(N, FA)
                ),
                "bonds": np.ascontiguousarray(
                    np.asarray(bonds[sl], dtype=np.float32).reshape(N, D * FB)
                ),
                "w": w_aug,
                "sampsel": sel,
            }
        )
    return in_maps


def mask_correction(atoms, bonds, edges, W, b):
    """Exact host-side correction: the device sums ALL rows; subtract the
    contribution of rows the reference masks out (edges all -1). Returns a
    [B, FP] f32 array (all zeros for the seeded inputs)."""
    edges = np.asarray(edges)
    mask_off = ~(edges != -1).any(-1)          # [B, A]
    corr = np.zeros((np.asarray(atoms).shape[0], FP), np.float32)
    if not mask_off.any():
        return corr
    bi, ai = np.nonzero(mask_off)
    at = np.asarray(atoms, np.float32)[bi, ai]             # [k, FA]
    bo = np.asarray(bonds, np.float32)[bi, ai].reshape(-1, D, FB).sum(1)
    cat = np.concatenate([at, bo], -1)
    fp = np.maximum(cat @ np.asarray(W, np.float32)
                    + np.asarray(b, np.float32).reshape(1, FP), 0.0)
    np.add.at(corr, bi, fp)
    return corr


def postprocess(concat_out):
    """Map concatenated per-core 'out' buffers to the full [B, FP] output."""
    return np.ascontiguousarray(np.asarray(concat_out).reshape(B, FP))


def kernel(atoms, bonds, edges, W, b):
    """Full inputs in, full output out. Shards batch across 8 cores."""
    global LAST_RESULTS
    BC = B // NCORES
    nc = build_nc(BC)
    in_maps = _shard_inputs(atoms, bonds, W, b, BC)
    core_ids = list(range(NCORES))
    trace = bool(os.environ.get("KERNEL_TRACE"))
    res = run_bass_kernel_spmd(nc, in_maps, core_ids, trace=trace)
    LAST_RESULTS = res
    out = np.concatenate([res.results[c]["out"] for c in range(NCORES)], axis=0)
    out = out.astype(np.float32) - mask_correction(atoms, bonds, edges, W, b)
    return out


# revision 42
# speedup vs baseline: 1.0586x; 1.0586x over previous
"""Trainium2 Bass kernel for NeuralGraphOutput (gnn_message_passing).

Math (per sample b):
    out[b, :] = sum_a mask[b,a] * relu(cat(atoms[b,a,:], sum_d bonds[b,a,d,:]) @ W + bias)
    mask[b,a] = any(edges[b,a,:] != -1)

For the fixed seeded inputs, mask is identically 1 (P(all 8 edges == -1) ~
(1/257)^8 per row); the device kernel computes the unmasked sum and the host
subtracts the (normally empty) set of masked-row contributions exactly.

Strategy: pure data parallel over 8 NeuronCores (256 samples each).
Rows (sample, atom pairs) are processed in blocks of 1024, mapped p-major:
row = blk*1024 + p*8 + g  (p = partition, g in [0,8)). This makes the atoms
DMA 2KB-contiguous and the bonds DMA 4KB-contiguous per partition (full DMA
line rate), with only 2 HWDGE dispatches per block.

Per block:
  - DMA atoms -> atile [128, 8*64] f32, bonds -> btile [128, 8*128] f32
  - GpSimd fold1: d-pairs (d, d+4) -> bt2 [128, 8, 4, 16] f32
  - GpSimd fold2: pairs -> cat[:, :, 64:96] f16  (2 groups of 4 d's each;
    W rows 64:80 / 80:96 are both W_bond so the group split is exact)
  - DVE casts atoms -> cat[:, :, 0:64] f16; cat cols 96 (=1.0 bias) and 97
    (=0 pad) are preset once per buffer
  - PE transposes cat[:, g, :] (f16, 1 cyc/row) -> psum_ct [98, 512]
  - DVE drains psum_ct -> catT [98, 512] f16
  - PE main matmuls: lhsT = catT[:, j*128:(j+1)*128] (stationary),
    rhs = W_aug [98, 256] f16 -> psum_fp [128, 4*256] per half-block
  - ACT relu psum_fp -> relu_sb [128, 4, 256] f16
  - PE reduce matmuls: lhsT = sampsel[:, r*16:(r+1)*16] (one-hot sample
    selector, r = blk%4), rhs = relu_sb[:, j, :] -> psum_out [16, 256]
    accumulated over 4 blocks (16 samples)
  - every 4 blocks: DVE drains psum_out -> stage, out DMA via the ACT
    HWDGE queue (keeps the SP queue clear for input loads); both are
    emitted 3 half-blocks late so their waits are already satisfied and
    they never head-of-line-block the DVE/ACT streams

Pipeline skew: transposes+drains run one block ahead of mains/relu, and
reduce matmuls trail by two half-blocks, so the in-order PE stream never
stalls on the ACT relu or the DVE drain.

Output is sample-major [256, 256] f32 per core; host concatenates.
"""

import os
from contextlib import ExitStack

import numpy as np

import concourse.bass as bass
import concourse.mybir as mybir
import concourse.tile as tile
from concourse import masks
from concourse.bass_utils import run_bass_kernel_spmd

# Problem shapes (hardcoded per contract)
B, A, D, FA, FB, FP = 2048, 256, 8, 64, 16, 256
NCORES = 8
P = 128
G = 8                      # rows per partition per block
ROWS = P * G               # rows per block (1024)
KC = 98                    # 64 atoms + 32 bond-halves + 1 bias + 1 pad
BPG = 4                    # blocks accumulated per psum_out group
SPG = 16                   # samples per group (BPG * ROWS // A)

f32 = mybir.dt.float32
f16 = mybir.dt.float16
f8 = mybir.dt.float8e4
i32 = mybir.dt.int32

# fp8e4m3 DoubleRow reduce matmuls: 2 sub-tiles per matmul at 0.5 cyc/row
REDUCE_FP8 = False


def _env(name, default):
    return int(os.environ.get(name, default))

# Set by kernel() after a run; test.py reads exec_time_ns / trace info.
LAST_RESULTS = None


def legalize_waits(nc, max_inline=1):
    """This toolchain's walrus accepts at most one semaphore wait inline per
    instruction (64B Events struct). Tile emits multi-wait sync_info; split
    the surplus into standalone EventSemaphore instructions just before the
    instruction on the same engine queue — identical semantics."""
    f = nc.m.functions[0]
    for bb in f.blocks:
        new = []
        for inst in bb.instructions:
            si = inst.sync_info
            waits = list(si.on_wait) if (si and si.on_wait) else []
            if len(waits) > max_inline:
                keep = waits[-max_inline:]
                moved = waits[:-max_inline]
                for k, w in enumerate(moved):
                    new.append(
                        mybir.InstEventSemaphore(
                            name=f"{inst.name}-prewait{k}",
                            ins=[],
                            outs=[],
                            sync_info=mybir.SyncInfo(on_wait=[w], on_update=[]),
                            engine=inst.engine,
                        )
                    )
                si.on_wait = keep
            new.append(inst)
        bb.instructions[:] = new
    return nc


def build_nc(n_samples_per_core: int, legalize: bool = True, repeat: int = 1) -> bass.Bass:
    """Build the single-core Bass program (same program runs SPMD on all cores).

    repeat > 1 duplicates the whole computation device-side (identical I/O) —
    used only for benchmarking, to amortize host/RPC dispatch overhead."""
    BC = n_samples_per_core
    N = BC * A                      # flat rows per core
    NBLK = N // ROWS                # 1024-row blocks
    NGRP = NBLK // BPG              # psum_out groups (16 samples each)
    assert NBLK % BPG == 0

    nc = bass.Bass()
    atoms_d = nc.dram_tensor("atoms", [N, FA], f32, kind="ExternalInput")
    bonds_d = nc.dram_tensor("bonds", [N, D * FB], f32, kind="ExternalInput")
    # host passes W_aug f16: rows 0:64 = W_atoms, 64:80 = W_bond, 80:96 =
    # W_bond (replicated for the 2 folded bond groups), 96 = bias, 97 = 0
    w_d = nc.dram_tensor("w", [KC, FP], f16, kind="ExternalInput")
    # sampsel[p, r*16+s] = 1 iff s == 4*r + p//32 (one-hot selector; for the
    # fp8 DoubleRow path it is duplicated across the 2 k-planes)
    sel_dt = f8 if REDUCE_FP8 else f16
    sel_cols = BPG * (2 * SPG if REDUCE_FP8 else SPG)
    sel_d = nc.dram_tensor("sampsel", [P, sel_cols], sel_dt, kind="ExternalInput")
    out_d = nc.dram_tensor("out", [BC, FP], f32, kind="ExternalOutput")

    with ExitStack() as ctx:
        tc = ctx.enter_context(tc_ := tile.TileContext(nc))
        singles = ctx.enter_context(tc.tile_pool(name="singles", bufs=1))

        # ---- pools ----
        atp = ctx.enter_context(tc.tile_pool(name="atp", bufs=_env("KATP", 3)))
        btp = ctx.enter_context(tc.tile_pool(name="btp", bufs=_env("KBTP", 3)))
        bt2p = ctx.enter_context(tc.tile_pool(name="bt2p", bufs=_env("KBT2", 3)))
        catTp = ctx.enter_context(tc.tile_pool(name="catTp", bufs=_env("KCTT", 4)))
        relup = ctx.enter_context(tc.tile_pool(name="relup", bufs=_env("KRELU", 4)))
        stagep = ctx.enter_context(tc.tile_pool(name="stagep", bufs=2))
        psct = ctx.enter_context(tc.tile_pool(name="psct", bufs=2, space="PSUM"))
        psfp = ctx.enter_context(tc.tile_pool(name="psfp", bufs=2, space="PSUM"))
        psout = ctx.enter_context(tc.tile_pool(name="psout", bufs=2, space="PSUM"))

        atoms_r = atoms_d[:, :].rearrange("(T p g) f -> T p (g f)", p=P, g=G)
        bonds_r = bonds_d[:, :].rearrange("(T p g) f -> T p (g f)", p=P, g=G)

        # block-0 loads issued before everything else: the first transposes
        # gate the whole pipeline on DMA + both gpsimd folds
        bt0 = btp.tile([P, G, D * FB], f32)
        nc.sync.dma_start(out=bt0[:], in_=bonds_r[0])
        at0 = atp.tile([P, G, FA], f32)
        nc.sync.dma_start(out=at0[:], in_=atoms_r[0])
        prefetched = {0: (at0, bt0)}

        # ---- constants ----
        w_sb = singles.tile([KC, FP], f16)
        nc.sync.dma_start(out=w_sb[:], in_=w_d[:, :])
        sel_sb = singles.tile([P, sel_cols], sel_dt)
        nc.sync.dma_start(out=sel_sb[:], in_=sel_d[:, :])
        # identity built on gpsimd (f32), laundered to f16 via DVE
        identity_src = singles.tile([P, P], f32)
        masks.make_identity(nc, identity_src[:])
        identity = singles.tile([P, P], f16)
        nc.vector.tensor_copy(identity[:], identity_src[:])

        # cat buffers: manual rotation; bias col 96 = 1.0 and pad col 97 = 0.0
        # are preset once per buffer and never rewritten
        NB = _env("KNB", 4)
        cat_bufs = []
        for i in range(NB):
            cb = singles.tile([P, G, KC], f16, name=f"cat{i}")
            nc.vector.memset(cb[:, :, FA + 2 * FB : FA + 2 * FB + 1], 1.0)
            nc.vector.memset(cb[:, :, FA + 2 * FB + 1 : KC], 0.0)
            cat_bufs.append(cb)

        # The PE sequencer is in-order: reduce matmuls for half-block h are
        # emitted two half-blocks late so they never head-of-line-block the
        # next halves' transposes/mains while ACT's relu is still running.
        pending = []        # [(relu, r, h, psum_out, grp_done: grp or None)]
        stage_q = []        # [(countdown_halves, grp, psum_out)]

        def emit_one():
            relu, r, h, pout, grp_done = pending.pop(0)
            if REDUCE_FP8:
                selv = sel_sb.rearrange("p (r e s) -> p r e s", r=BPG, e=2)
                reluv = relu.rearrange("p (u e) o -> p u e o", e=2)
                for j in range(2):
                    t = 4 * r + 2 * h + j
                    nc.tensor.matmul(
                        pout[:, :],
                        lhsT=selv[:, r],
                        rhs=reluv[:, j],
                        start=(t == 0),
                        stop=(t == 4 * BPG - 1),
                        perf_mode=mybir.MatmulPerfMode.DoubleRow,
                    )
            else:
                for j in range(4):
                    t = 8 * r + 4 * h + j
                    nc.tensor.matmul(
                        pout[:, :],
                        lhsT=sel_sb[:, r * SPG : (r + 1) * SPG],
                        rhs=relu[:, j, :],
                        start=(t == 0),
                        stop=(t == 8 * BPG - 1),
                    )
            if grp_done is not None:
                stage_q.append([_env("KSTG", 3), grp_done, pout])

        def tick_stage(force=False):
            # Emit the psum_out drain only once its producer reduces are
            # long finished, so it never head-of-line-blocks the DVE stream;
            # the out DMA goes through SWDGE to keep the SP queue clear.
            for ent in stage_q:
                ent[0] -= 1
            while stage_q and (force or stage_q[0][0] <= 0):
                _, grp, pout = stage_q.pop(0)
                stage = stagep.tile([SPG, FP], f32)
                nc.vector.tensor_copy(stage[:], pout[:])
                nc.scalar.dma_start(
                    out=out_d[grp * SPG : (grp + 1) * SPG, :], in_=stage[:]
                )

        psum_out = None
        catT_q = {}          # blk -> [catT_h0, catT_h1], transposed 1 blk early
        for rep_blk in range(repeat * (NBLK + 1)):
            blk = rep_blk % (NBLK + 1)
            if blk < NBLK:
                cat = cat_bufs[blk % NB]
                if blk in prefetched:
                    atile, btile = prefetched.pop(blk)
                else:
                    btile = btp.tile([P, G, D * FB], f32)
                    nc.sync.dma_start(out=btile[:], in_=bonds_r[blk])
                    atile = atp.tile([P, G, FA], f32)
                    nc.sync.dma_start(out=atile[:], in_=atoms_r[blk])

                # bond fold on gpsimd: (d, d+4) pairs, then pairs again
                bv = btile.rearrange("p g (e x) -> p g e x", e=2)
                bt2 = bt2p.tile([P, G, (D // 2) * FB], f32)
                nc.gpsimd.tensor_tensor(
                    out=bt2[:], in0=bv[:, :, 0], in1=bv[:, :, 1],
                    op=mybir.AluOpType.add,
                )
                b2v = bt2.rearrange("p g (e x) -> p g e x", e=2)
                nc.gpsimd.tensor_tensor(
                    out=cat[:, :, FA : FA + 2 * FB],
                    in0=b2v[:, :, 0], in1=b2v[:, :, 1],
                    op=mybir.AluOpType.add,
                )
                # atoms cast f32 -> f16 into cat
                nc.vector.tensor_scalar(
                    out=cat[:, :, 0:FA], in0=atile[:], scalar1=0.0,
                    scalar2=None, op0=mybir.AluOpType.add,
                )

                # stage A: transpose + drain (one block ahead of stage B, so
                # the DVE drains overlap the previous block's mains/reduces)
                cts = []
                for h in range(2):
                    psum_ct = psct.tile([KC, 4 * P], f16)
                    for j in range(4):
                        nc.tensor.transpose(
                            psum_ct[:, j * P : (j + 1) * P],
                            cat[:, 4 * h + j, :],
                            identity[:],
                        )
                    catT = catTp.tile([KC, 4 * P], f16)
                    nc.vector.tensor_copy(catT[:], psum_ct[:, :])
                    cts.append(catT)
                catT_q[blk] = cts

            # stage B/C: mains + relu for block blk-1, reduces one half late
            mblk = blk - 1
            if mblk < 0:
                continue
            r = mblk % BPG
            if r == 0:
                psum_out = psout.tile([SPG, FP], f32, name="psum_out")
            for h in range(2):
                catT = catT_q[mblk][h]
                psum_fp = psfp.tile([P, 4 * FP], f32)
                for j in range(4):
                    nc.tensor.matmul(
                        psum_fp[:, j * FP : (j + 1) * FP],
                        lhsT=catT[:, j * P : (j + 1) * P],
                        rhs=w_sb[:, :],
                        start=True,
                        stop=True,
                    )
                relu = relup.tile([P, 4, FP], f8 if REDUCE_FP8 else f16)
                pfv = psum_fp[:].rearrange("p (j o) -> p j o", j=4)
                NGA = _env("KNGA", 4)   # relu g-slices on ACT; rest on DVE
                nc.scalar.activation(
                    relu[:, 0:NGA], pfv[:, 0:NGA],
                    mybir.ActivationFunctionType.Relu,
                )
                if NGA < 4:
                    nc.vector.tensor_scalar(
                        out=relu[:, NGA:4], in0=pfv[:, NGA:4], scalar1=0.0,
                        scalar2=None, op0=mybir.AluOpType.max,
                    )
                grp_done = mblk // BPG if (r == BPG - 1 and h == 1) else None
                pending.append((relu, r, h, psum_out, grp_done))
                if len(pending) > _env("KPEND", 2):
                    emit_one()
                tick_stage()
            del catT_q[mblk]
            if blk == NBLK:
                while pending:
                    emit_one()
                tick_stage(force=True)

        while pending:
            emit_one()
        tick_stage(force=True)
    if legalize:
        legalize_waits(nc)
    return nc


def make_w_aug(W, b):
    """Host-side W layout matching cat columns: atoms | bond x2 | bias | 0."""
    W = np.asarray(W, dtype=np.float32)
    b = np.asarray(b, dtype=np.float32).reshape(1, FP)
    rows = [W[0:FA], W[FA : FA + FB], W[FA : FA + FB], b,
            np.zeros((1, FP), np.float32)]
    return np.ascontiguousarray(np.vstack(rows)).astype(np.float16)


def make_sampsel():
    """sampsel[p, r*16+s] = 1 iff s == 4*r + p//32 (fp8 path: duplicated
    across the 2 DoubleRow k-planes)."""
    if REDUCE_FP8:
        sel = np.zeros((P, BPG, 2, SPG), np.float32)
        for p in range(P):
            for r in range(BPG):
                sel[p, r, :, 4 * r + p // 32] = 1.0
        return sel.reshape(P, -1).astype(mybir.dt.np(f8))
    sel = np.zeros((P, BPG * SPG), np.float16)
    for p in range(P):
        for r in range(BPG):
            sel[p, r * SPG + 4 * r + p // 32] = 1.0
    return sel


def _shard_inputs(atoms, bonds, W, b, n_samples_per_core):
    BC = n_samples_per_core
    N = BC * A
    w_aug = make_w_aug(W, b)
    sel = make_sampsel()
    in_maps = []
    for c in range(NCORES):
        sl = slice(c * BC, (c + 1) * BC)
        in_maps.append(
            {
                "atoms": np.ascontiguousarray(
                    np.asarray(atoms[sl], dtype=np.float32).reshape(N, FA)
                ),
                "bonds": np.ascontiguousarray(
                    np.asarray(bonds[sl], dtype=np.float32).reshape(N, D * FB)
                ),
                "w": w_aug,
                "sampsel": sel,
            }
        )
    return in_maps


def mask_correction(atoms, bonds, edges, W, b):
    """Exact host-side correction: the device sums ALL rows; subtract the
    contribution of rows the reference masks out (edges all -1). Returns a
    [B, FP] f32 array (all zeros for the seeded inputs)."""
    edges = np.asarray(edges)
    mask_off = ~(edges != -1).any(-1)          # [B, A]
    corr = np.zeros((np.asarray(atoms).shape[0], FP), np.float32)
    if not mask_off.any():
        return corr
    bi, ai = np.nonzero(mask_off)
    at = np.asarray(atoms, np.float32)[bi, ai]             # [k, FA]
    bo = np.asarray(bonds, np.float32)[bi, ai].reshape(-1, D, FB).sum(1)
    cat = np.concatenate([at, bo], -1)
    fp = np.maximum(cat @ np.asarray(W, np.float32)
                    + np.asarray(b, np.float32).reshape(1, FP), 0.0)
    np.add.at(corr, bi, fp)
    return corr


def postprocess(concat_out):
    """Map concatenated per-core 'out' buffers to the full [B, FP] output."""
    return np.ascontiguousarray(np.asarray(concat_out).reshape(B, FP))


def kernel(atoms, bonds, edges, W, b):
    """Full inputs in, full output out. Shards batch across 8 cores."""
    global LAST_RESULTS
    BC = B // NCORES
    nc = build_nc(BC)
    in_maps = _shard_inputs(atoms, bonds, W, b, BC)
    core_ids = list(range(NCORES))
    trace = bool(os.environ.get("KERNEL_TRACE"))
    res = run_bass_kernel_spmd(nc, in_maps, core_ids, trace=trace)
    LAST_RESULTS = res
    out = np.concatenate([res.results[c]["out"] for c in range(NCORES)], axis=0)
    out = out.astype(np.float32) - mask_correction(atoms, bonds, edges, W, b)
    return out


# revision 43
# speedup vs baseline: 1.3114x; 1.2389x over previous
"""Trainium2 Bass kernel for NeuralGraphOutput (gnn_message_passing).

Math (per sample b):
    out[b, :] = sum_a mask[b,a] * relu(cat(atoms[b,a,:], sum_d bonds[b,a,d,:]) @ W + bias)
    mask[b,a] = any(edges[b,a,:] != -1)

For the fixed seeded inputs, mask is identically 1 (P(all 8 edges == -1) ~
(1/257)^8 per row); the device kernel computes the unmasked sum and the host
subtracts the (normally empty) set of masked-row contributions exactly.

Strategy: pure data parallel over 8 NeuronCores (256 samples each).
Rows (sample, atom pairs) are processed in blocks of 1024, mapped p-major:
row = blk*1024 + p*8 + g  (p = partition, g in [0,8)). This makes the atoms
DMA 2KB-contiguous and the bonds DMA 4KB-contiguous per partition (full DMA
line rate), with only 2 HWDGE dispatches per block.

Per block:
  - DMA atoms -> atile [128, 8*64] f32, bonds -> btile [128, 8*128] f32
  - GpSimd fold1: d-pairs (d, d+4) -> bt2 [128, 8, 4, 16] f32
  - GpSimd fold2: pairs -> cat[:, :, 64:96] f16  (2 groups of 4 d's each;
    W rows 64:80 / 80:96 are both W_bond so the group split is exact)
  - DVE casts atoms -> cat[:, :, 0:64] f16; cat cols 96 (=1.0 bias) and 97
    (=0 pad) are preset once per buffer
  - PE transposes cat[:, g, :] (f16, 1 cyc/row) -> psum_ct [98, 512]
  - DVE drains psum_ct -> catT [98, 512] f16
  - PE main matmuls: lhsT = catT[:, j*128:(j+1)*128] (stationary),
    rhs = W_aug [98, 256] f16 -> psum_fp [128, 4*256] per half-block
  - ACT relu psum_fp -> relu_sb [128, 4, 256] f16
  - PE reduce matmuls: lhsT = sampsel[:, r*16:(r+1)*16] (one-hot sample
    selector, r = blk%4), rhs = relu_sb[:, j, :] -> psum_out [16, 256]
    accumulated over 4 blocks (16 samples)
  - every 4 blocks: DVE drains psum_out -> stage, out DMA via the ACT
    HWDGE queue (keeps the SP queue clear for input loads); both are
    emitted 3 half-blocks late so their waits are already satisfied and
    they never head-of-line-block the DVE/ACT streams

Pipeline skew: transposes+drains run one block ahead of mains/relu, and
reduce matmuls trail by two half-blocks, so the in-order PE stream never
stalls on the ACT relu or the DVE drain.

Output is sample-major [256, 256] f32 per core; host concatenates.
"""

import os
from contextlib import ExitStack

import numpy as np

import concourse.bass as bass
import concourse.mybir as mybir
import concourse.tile as tile
from concourse import masks
from concourse.bass_utils import run_bass_kernel_spmd

# Problem shapes (hardcoded per contract)
B, A, D, FA, FB, FP = 2048, 256, 8, 64, 16, 256
NCORES = 8
P = 128
G = 8                      # rows per partition per block
ROWS = P * G               # rows per block (1024)
KC = 98                    # 64 atoms + 32 bond-halves + 1 bias + 1 pad
BPG = 4                    # blocks accumulated per psum_out group
SPG = 16                   # samples per group (BPG * ROWS // A)

f32 = mybir.dt.float32
f16 = mybir.dt.float16
f8 = mybir.dt.float8e4
i32 = mybir.dt.int32

# fp8e4m3 DoubleRow reduce matmuls: 2 sub-tiles per matmul at 0.5 cyc/row
REDUCE_FP8 = False


def _env(name, default):
    return int(os.environ.get(name, default))

# Set by kernel() after a run; test.py reads exec_time_ns / trace info.
LAST_RESULTS = None


def legalize_waits(nc, max_inline=1):
    """This toolchain's walrus accepts at most one semaphore wait inline per
    instruction (64B Events struct). Tile emits multi-wait sync_info; split
    the surplus into standalone EventSemaphore instructions just before the
    instruction on the same engine queue — identical semantics."""
    f = nc.m.functions[0]
    for bb in f.blocks:
        new = []
        for inst in bb.instructions:
            si = inst.sync_info
            waits = list(si.on_wait) if (si and si.on_wait) else []
            if len(waits) > max_inline:
                keep = waits[-max_inline:]
                moved = waits[:-max_inline]
                for k, w in enumerate(moved):
                    new.append(
                        mybir.InstEventSemaphore(
                            name=f"{inst.name}-prewait{k}",
                            ins=[],
                            outs=[],
                            sync_info=mybir.SyncInfo(on_wait=[w], on_update=[]),
                            engine=inst.engine,
                        )
                    )
                si.on_wait = keep
            new.append(inst)
        bb.instructions[:] = new
    return nc


def build_nc(n_samples_per_core: int, legalize: bool = True, repeat: int = 1) -> bass.Bass:
    """Build the single-core Bass program (same program runs SPMD on all cores).

    repeat > 1 duplicates the whole computation device-side (identical I/O) —
    used only for benchmarking, to amortize host/RPC dispatch overhead."""
    BC = n_samples_per_core
    N = BC * A                      # flat rows per core
    NBLK = N // ROWS                # 1024-row blocks
    NGRP = NBLK // BPG              # psum_out groups (16 samples each)
    assert NBLK % BPG == 0

    nc = bass.Bass()
    atoms_d = nc.dram_tensor("atoms", [N, FA], f32, kind="ExternalInput")
    bonds_d = nc.dram_tensor("bonds", [N, D * FB], f32, kind="ExternalInput")
    # host passes W_aug f16: rows 0:64 = W_atoms, 64:80 = W_bond, 80:96 =
    # W_bond (replicated for the 2 folded bond groups), 96 = bias, 97 = 0
    w_d = nc.dram_tensor("w", [KC, FP], f16, kind="ExternalInput")
    # sampsel[p, r*16+s] = 1 iff s == 4*r + p//32 (one-hot selector; for the
    # fp8 DoubleRow path it is duplicated across the 2 k-planes)
    sel_dt = f8 if REDUCE_FP8 else f16
    sel_cols = BPG * (2 * SPG if REDUCE_FP8 else SPG)
    sel_d = nc.dram_tensor("sampsel", [P, sel_cols], sel_dt, kind="ExternalInput")
    out_d = nc.dram_tensor("out", [BC, FP], f32, kind="ExternalOutput")

    with ExitStack() as ctx:
        tc = ctx.enter_context(tc_ := tile.TileContext(nc))
        singles = ctx.enter_context(tc.tile_pool(name="singles", bufs=1))

        # ---- pools ----
        atp = ctx.enter_context(tc.tile_pool(name="atp", bufs=_env("KATP", 3)))
        btp = ctx.enter_context(tc.tile_pool(name="btp", bufs=_env("KBTP", 3)))
        bt2p = ctx.enter_context(tc.tile_pool(name="bt2p", bufs=_env("KBT2", 3)))
        catTp = ctx.enter_context(tc.tile_pool(name="catTp", bufs=_env("KCTT", 4)))
        relup = ctx.enter_context(tc.tile_pool(name="relup", bufs=_env("KRELU", 4)))
        stagep = ctx.enter_context(tc.tile_pool(name="stagep", bufs=2))
        KWIDE = _env("KWIDE", 0)
        # KWIDE: one [98, 1024] f16 psct tile per block (1 bank) + single
        # drain, freeing banks for psfp bufs=3 (decouples mains from relu)
        psct = ctx.enter_context(tc.tile_pool(
            name="psct", bufs=(1 if KWIDE else 2), space="PSUM"))
        psfp = ctx.enter_context(tc.tile_pool(
            name="psfp", bufs=(3 if KWIDE else 2), space="PSUM"))
        psout = ctx.enter_context(tc.tile_pool(
            name="psout", bufs=(1 if KWIDE else 2), space="PSUM"))

        atoms_r = atoms_d[:, :].rearrange("(T p g) f -> T p (g f)", p=P, g=G)
        bonds_r = bonds_d[:, :].rearrange("(T p g) f -> T p (g f)", p=P, g=G)

        # block-0 loads issued before everything else: the first transposes
        # gate the whole pipeline on DMA + both gpsimd folds
        bt0 = btp.tile([P, G, D * FB], f32)
        nc.sync.dma_start(out=bt0[:], in_=bonds_r[0])
        at0 = atp.tile([P, G, FA], f32)
        nc.sync.dma_start(out=at0[:], in_=atoms_r[0])
        prefetched = {0: (at0, bt0)}

        # ---- constants ----
        w_sb = singles.tile([KC, FP], f16)
        nc.sync.dma_start(out=w_sb[:], in_=w_d[:, :])
        sel_sb = singles.tile([P, sel_cols], sel_dt)
        nc.sync.dma_start(out=sel_sb[:], in_=sel_d[:, :])
        # identity built on gpsimd (f32), laundered to f16 via DVE
        identity_src = singles.tile([P, P], f32)
        masks.make_identity(nc, identity_src[:])
        identity = singles.tile([P, P], f16)
        nc.vector.tensor_copy(identity[:], identity_src[:])

        # cat buffers: manual rotation; bias col 96 = 1.0 and pad col 97 = 0.0
        # are preset once per buffer and never rewritten
        NB = _env("KNB", 4)
        cat_bufs = []
        for i in range(NB):
            cb = singles.tile([P, G, KC], f16, name=f"cat{i}")
            nc.vector.memset(cb[:, :, FA + 2 * FB : FA + 2 * FB + 1], 1.0)
            nc.vector.memset(cb[:, :, FA + 2 * FB + 1 : KC], 0.0)
            cat_bufs.append(cb)

        # The PE sequencer is in-order: reduce matmuls for half-block h are
        # emitted two half-blocks late so they never head-of-line-block the
        # next halves' transposes/mains while ACT's relu is still running.
        pending = []        # [(relu, r, h, psum_out, grp_done: grp or None)]
        stage_q = []        # [(countdown_halves, grp, psum_out)]

        def emit_one():
            relu, r, h, pout, grp_done = pending.pop(0)
            if REDUCE_FP8:
                selv = sel_sb.rearrange("p (r e s) -> p r e s", r=BPG, e=2)
                reluv = relu.rearrange("p (u e) o -> p u e o", e=2)
                for j in range(2):
                    t = 4 * r + 2 * h + j
                    nc.tensor.matmul(
                        pout[:, :],
                        lhsT=selv[:, r],
                        rhs=reluv[:, j],
                        start=(t == 0),
                        stop=(t == 4 * BPG - 1),
                        perf_mode=mybir.MatmulPerfMode.DoubleRow,
                    )
            else:
                for j in range(4):
                    t = 8 * r + 4 * h + j
                    nc.tensor.matmul(
                        pout[:, :],
                        lhsT=sel_sb[:, r * SPG : (r + 1) * SPG],
                        rhs=relu[:, j, :],
                        start=(t == 0),
                        stop=(t == 8 * BPG - 1),
                    )
            if grp_done is not None:
                stage_q.append([_env("KSTG", 3), grp_done, pout])

        def tick_stage(force=False):
            # Emit the psum_out drain only once its producer reduces are
            # long finished, so it never head-of-line-blocks the DVE stream;
            # the out DMA goes through SWDGE to keep the SP queue clear.
            for ent in stage_q:
                ent[0] -= 1
            while stage_q and (force or stage_q[0][0] <= 0):
                _, grp, pout = stage_q.pop(0)
                stage = stagep.tile([SPG, FP], f32)
                nc.vector.tensor_copy(stage[:], pout[:])
                nc.scalar.dma_start(
                    out=out_d[grp * SPG : (grp + 1) * SPG, :], in_=stage[:]
                )

        psum_out = None
        catT_q = {}          # blk -> [catT_h0, catT_h1], transposed 1 blk early
        for rep_blk in range(repeat * (NBLK + 1)):
            blk = rep_blk % (NBLK + 1)
            if blk < NBLK:
                cat = cat_bufs[blk % NB]
                if blk in prefetched:
                    atile, btile = prefetched.pop(blk)
                else:
                    btile = btp.tile([P, G, D * FB], f32)
                    nc.sync.dma_start(out=btile[:], in_=bonds_r[blk])
                    atile = atp.tile([P, G, FA], f32)
                    nc.sync.dma_start(out=atile[:], in_=atoms_r[blk])

                # bond fold on gpsimd: (d, d+4) pairs, then pairs again
                bv = btile.rearrange("p g (e x) -> p g e x", e=2)
                bt2 = bt2p.tile([P, G, (D // 2) * FB], f32)
                nc.gpsimd.tensor_tensor(
                    out=bt2[:], in0=bv[:, :, 0], in1=bv[:, :, 1],
                    op=mybir.AluOpType.add,
                )
                b2v = bt2.rearrange("p g (e x) -> p g e x", e=2)
                nc.gpsimd.tensor_tensor(
                    out=cat[:, :, FA : FA + 2 * FB],
                    in0=b2v[:, :, 0], in1=b2v[:, :, 1],
                    op=mybir.AluOpType.add,
                )
                # atoms cast f32 -> f16 into cat
                nc.vector.tensor_scalar(
                    out=cat[:, :, 0:FA], in0=atile[:], scalar1=0.0,
                    scalar2=None, op0=mybir.AluOpType.add,
                )

                # stage A: transpose + drain (one block ahead of stage B, so
                # the DVE drains overlap the previous block's mains/reduces)
                if KWIDE:
                    psum_ct = psct.tile([KC, G * P], f16)
                    for g in range(G):
                        nc.tensor.transpose(
                            psum_ct[:, g * P : (g + 1) * P],
                            cat[:, g, :],
                            identity[:],
                        )
                    catT = catTp.tile([KC, G * P], f16)
                    nc.vector.tensor_copy(catT[:], psum_ct[:, :])
                    catT_q[blk] = [
                        catT[:, 0 : 4 * P], catT[:, 4 * P : 8 * P]
                    ]
                else:
                    cts = []
                    for h in range(2):
                        psum_ct = psct.tile([KC, 4 * P], f16)
                        for j in range(4):
                            nc.tensor.transpose(
                                psum_ct[:, j * P : (j + 1) * P],
                                cat[:, 4 * h + j, :],
                                identity[:],
                            )
                        catT = catTp.tile([KC, 4 * P], f16)
                        nc.vector.tensor_copy(catT[:], psum_ct[:, :])
                        cts.append(catT)
                    catT_q[blk] = cts

            # stage B/C: mains + relu for block blk-1, reduces one half late
            mblk = blk - 1
            if mblk < 0:
                continue
            r = mblk % BPG
            if r == 0:
                psum_out = psout.tile([SPG, FP], f32, name="psum_out")
            for h in range(2):
                catT = catT_q[mblk][h]
                psum_fp = psfp.tile([P, 4 * FP], f32)
                for j in range(4):
                    nc.tensor.matmul(
                        psum_fp[:, j * FP : (j + 1) * FP],
                        lhsT=catT[:, j * P : (j + 1) * P],
                        rhs=w_sb[:, :],
                        start=True,
                        stop=True,
                    )
                relu = relup.tile([P, 4, FP], f8 if REDUCE_FP8 else f16)
                pfv = psum_fp[:].rearrange("p (j o) -> p j o", j=4)
                NGA = _env("KNGA", 4)   # relu g-slices on ACT; rest on DVE
                nc.scalar.activation(
                    relu[:, 0:NGA], pfv[:, 0:NGA],
                    mybir.ActivationFunctionType.Relu,
                )
                if NGA < 4:
                    nc.vector.tensor_scalar(
                        out=relu[:, NGA:4], in0=pfv[:, NGA:4], scalar1=0.0,
                        scalar2=None, op0=mybir.AluOpType.max,
                    )
                grp_done = mblk // BPG if (r == BPG - 1 and h == 1) else None
                pending.append((relu, r, h, psum_out, grp_done))
                if len(pending) > _env("KPEND", 2):
                    emit_one()
                tick_stage()
            del catT_q[mblk]
            if blk == NBLK:
                while pending:
                    emit_one()
                tick_stage(force=True)

        while pending:
            emit_one()
        tick_stage(force=True)
    if legalize:
        legalize_waits(nc)
    return nc


def make_w_aug(W, b):
    """Host-side W layout matching cat columns: atoms | bond x2 | bias | 0."""
    W = np.asarray(W, dtype=np.float32)
    b = np.asarray(b, dtype=np.float32).reshape(1, FP)
    rows = [W[0:FA], W[FA : FA + FB], W[FA : FA + FB], b,
            np.zeros((1, FP), np.float32)]
    return np.ascontiguousarray(np.vstack(rows)).astype(np.float16)


def make_sampsel():
    """sampsel[p, r*16+s] = 1 iff s == 4*r + p//32 (fp8 path: duplicated
    across the 2 DoubleRow k-planes)."""
    if REDUCE_FP8:
        sel = np.zeros((P, BPG, 2, SPG), np.float32)
        for p in range(P):
            for r in range(BPG):
                sel[p, r, :, 4 * r + p // 32] = 1.0
        return sel.reshape(P, -1).astype(mybir.dt.np(f8))
    sel = np.zeros((P, BPG * SPG), np.float16)
    for p in range(P):
        for r in range(BPG):
            sel[p, r * SPG + 4 * r + p // 32] = 1.0
    return sel


def _shard_inputs(atoms, bonds, W, b, n_samples_per_core):
    BC = n_samples_per_core
    N = BC * A
    w_aug = make_w_aug(W, b)
    sel = make_sampsel()
    in_maps = []
    for c in range(NCORES):
        sl = slice(c * BC, (c + 1) * BC)
        in_maps.append(
            {
                "atoms": np.ascontiguousarray(
                    np.asarray(atoms[sl], dtype=np.float32).reshape(N, FA)
                ),
                "bonds": np.ascontiguousarray(
                    np.asarray(bonds[sl], dtype=np.float32).reshape(N, D * FB)
                ),
                "w": w_aug,
                "sampsel": sel,
            }
        )
    return in_maps


def mask_correction(atoms, bonds, edges, W, b):
    """Exact host-side correction: the device sums ALL rows; subtract the
    contribution of rows the reference masks out (edges all -1). Returns a
    [B, FP] f32 array (all zeros for the seeded inputs)."""
    edges = np.asarray(edges)
    mask_off = ~(edges != -1).any(-1)          # [B, A]
    corr = np.zeros((np.asarray(atoms).shape[0], FP), np.float32)
    if not mask_off.any():
        return corr
    bi, ai = np.nonzero(mask_off)
    at = np.asarray(atoms, np.float32)[bi, ai]             # [k, FA]
    bo = np.asarray(bonds, np.float32)[bi, ai].reshape(-1, D, FB).sum(1)
    cat = np.concatenate([at, bo], -1)
    fp = np.maximum(cat @ np.asarray(W, np.float32)
                    + np.asarray(b, np.float32).reshape(1, FP), 0.0)
    np.add.at(corr, bi, fp)
    return corr


def postprocess(concat_out):
    """Map concatenated per-core 'out' buffers to the full [B, FP] output."""
    return np.ascontiguousarray(np.asarray(concat_out).reshape(B, FP))


def kernel(atoms, bonds, edges, W, b):
    """Full inputs in, full output out. Shards batch across 8 cores."""
    global LAST_RESULTS
    BC = B // NCORES
    nc = build_nc(BC)
    in_maps = _shard_inputs(atoms, bonds, W, b, BC)
    core_ids = list(range(NCORES))
    trace = bool(os.environ.get("KERNEL_TRACE"))
    res = run_bass_kernel_spmd(nc, in_maps, core_ids, trace=trace)
    LAST_RESULTS = res
    out = np.concatenate([res.results[c]["out"] for c in range(NCORES)], axis=0)
    out = out.astype(np.float32) - mask_correction(atoms, bonds, edges, W, b)
    return out
